# revision 3
# baseline (speedup 1.0000x reference)
"""MultiHeadAttention (B=4, S=2048, D=2048, H=16) on 8 TRN2 NeuronCores.

Sharding: core c handles batch b = c//2 and head-half = c%2 (8 heads).
Permuted-coordinate layout trick as v1-v3 (pi = 128*t + u, s = 16*u + t);
host sums two partial output projections per batch and un-permutes.

v4 (over v3): prefetch DMAs (wv weight blocks, wo tiles) are
dependency-gated behind projection evacuations so the critical first
stage half owns the DMA bandwidth (PE starts ~5us in); Q projection's
rb=0 half is interleaved into attention j=0 (head order 4..7,0..3 per j)
using the O-projection PSUM bank, so every attention block has matmul
filler; x is double-buffered across j; output partials are written bf16
and summed in f32 on the host.
"""
import math
from contextlib import ExitStack

import numpy as np

B, S, D, H = 4, 2048, 2048, 16
DK = D // H            # 128
HPC = H // 2           # heads per core = 8
RPC = HPC * DK         # rows per core = 1024
NC_ = 8                # cores
MC = D // 128          # contraction chunks = 16
NP_ = MC // 2          # key-tile pairs per head-block = 8
SCALE = 1.0 / math.sqrt(DK)

_cache = {}
last_results = None


def _bf16():
    import ml_dtypes

    return ml_dtypes.bfloat16


def _build():
    import concourse.bass as bass
    import concourse.mybir as mybir
    import concourse.tile as tile
    from concourse import bacc

    f32 = mybir.dt.float32
    bf16 = mybir.dt.bfloat16
    AF = mybir.ActivationFunctionType

    def gate(waiter_inst, dep_inst, why):
        if waiter_inst is not None and dep_inst is not None:
            tile.add_dep_helper(waiter_inst.ins, dep_inst.ins, sync=True,
                                reason=why)

    nc = bacc.Bacc("TRN2", target_bir_lowering=False, debug=False,
                   num_devices=NC_)

    kts_d = nc.dram_tensor("kts", (MC, 128, RPC), bf16, kind="ExternalInput")
    vts_d = nc.dram_tensor("vts", (MC, 128, RPC), bf16, kind="ExternalInput")
    qts_d = nc.dram_tensor("qts", (MC, 128, RPC), bf16, kind="ExternalInput")
    wkt_d = nc.dram_tensor("wkt", (MC, 128, MC, 128), bf16, kind="ExternalInput")
    wqt_d = nc.dram_tensor("wqt", (MC, 128, MC, 128), bf16, kind="ExternalInput")
    wvt_d = nc.dram_tensor("wvt", (4, 128, MC, 512), bf16, kind="ExternalInput")
    wot_d = nc.dram_tensor("wot", (MC, 128, HPC, 128), bf16, kind="ExternalInput")
    bias_d = nc.dram_tensor("bias", (128, 3 * MC), f32, kind="ExternalInput")
    bvr_d = nc.dram_tensor("bvr", (1, D), f32, kind="ExternalInput")
    out_d = nc.dram_tensor("out", (D, S), bf16, kind="ExternalOutput")

    def half_src(src_d, half, m0, m1):
        return (src_d.ap()[m0:m1, :, 512 * half:512 * half + 512]
                .rearrange("m p r -> p m r"))

    with tile.TileContext(nc) as tc, ExitStack() as top:
        cpool = top.enter_context(tc.tile_pool(name="consts", bufs=1))
        rpool = top.enter_context(tc.tile_pool(name="resident", bufs=1))

        khat = rpool.tile([128, MC, HPC, 128], bf16)   # [d'][tk][h][u]
        vhat = rpool.tile([128, HPC, MC, 128], bf16)   # [u][h][tk][d']
        q_sb = rpool.tile([128, MC, HPC, 128], bf16)   # [d'][tq][h][u]

        bias_sb = cpool.tile([128, 3 * MC], f32)
        bq_sb = bias_sb[:, 0:MC]
        bk_sb = bias_sb[:, MC:2 * MC]
        bo_sb = bias_sb[:, 2 * MC:3 * MC]
        onescol = cpool.tile([128, 1], bf16)
        nc.vector.memset(onescol[:], 1.0)

        with ExitStack() as outer:
            st_pool = outer.enter_context(tc.tile_pool(name="stages", bufs=2))

            # K/V evacuation instructions, for gating prefetch DMAs
            kevac = {}
            vevac = {}
            qevac = {}

            with ExitStack() as projes:
                w_pool = projes.enter_context(tc.tile_pool(name="wkq", bufs=4))
                wv_pool = projes.enter_context(tc.tile_pool(name="wv", bufs=2))
                bv_pool = projes.enter_context(tc.tile_pool(name="bvp", bufs=1))
                pps_pool = projes.enter_context(
                    tc.tile_pool(name="pps", bufs=4, space="PSUM"))
                bvr_sb = bv_pool.tile([1, D], f32)
                bvb = bv_pool.tile([128, D], f32)

                # first weight chunk first; first stage half split so the
                # first psum group can start on chunks 0-3
                wk0 = w_pool.tile([128, MC, 128], bf16, tag="w")
                nc.sync.dma_start(wk0[:], wkt_d.ap()[0])
                kt_h = []
                st0 = st_pool.tile([128, MC, 512], bf16, tag="st")
                nc.sync.dma_start(st0[:, 0:4, :], half_src(kts_d, 0, 0, 4))
                nc.sync.dma_start(st0[:, 4:MC, :], half_src(kts_d, 0, 4, MC))
                kt_h.append(st0)
                st1 = st_pool.tile([128, MC, 512], bf16, tag="st")
                kt1_dma = nc.sync.dma_start(st1[:], half_src(kts_d, 1, 0, MC))
                kt_h.append(st1)
                nc.sync.dma_start(bias_sb[:], bias_d.ap())
                nc.sync.dma_start(bvr_sb[:], bvr_d.ap())
                nc.gpsimd.partition_broadcast(bvb[:], bvr_sb[:])

                # ---- K projection -> khat ----
                with nc.named_scope("proj_k"):
                    for rb in range(2):
                        for ct in range(MC):
                            if rb == 0 and ct == 0:
                                wk_st = wk0
                            else:
                                wk_st = w_pool.tile([128, MC, 128], bf16,
                                                    tag="w")
                                nc.sync.dma_start(wk_st[:], wkt_d.ap()[ct])
                            ps = pps_pool.tile([128, 512], f32, tag="ps")
                            for mc in range(MC):
                                nc.tensor.matmul(
                                    ps[:], wk_st[:, mc, :], kt_h[rb][:, mc, :],
                                    start=(mc == 0), stop=(mc == MC - 1))
                            kevac[(rb, ct)] = nc.scalar.activation(
                                khat[:, ct, 4 * rb:4 * rb + 4, :], ps[:],
                                AF.Identity, bias=bk_sb[:, ct:ct + 1],
                                scale=1.0)
                # keep kt half1 off the startup DMA window
                gate(kt1_dma, kevac.get((0, 6)), "kt1 after early K evac")

                # ---- V projection -> vhat (bias via DVE broadcast add) ----
                with nc.named_scope("proj_v"):
                    vt_h = []
                    for half in range(2):
                        st = st_pool.tile([128, MC, 512], bf16, tag="st")
                        nc.sync.dma_start(st[:], half_src(vts_d, half, 0, MC))
                        vt_h.append(st)
                    for half in range(2):
                        for cb in range(4):
                            wv_st = wv_pool.tile([128, MC, 512], bf16,
                                                 tag="wv")
                            wv_dma = nc.sync.dma_start(wv_st[:],
                                                       wvt_d.ap()[cb])
                            if half == 0:
                                gate(wv_dma, kevac[(1, 4 * cb)],
                                     "wv prefetch after late K evac")
                            else:
                                gate(wv_dma, vevac[(0, cb, 3)],
                                     "wv half1 prefetch after V evac")
                            for hl in range(4):
                                h = 4 * half + hl
                                ps = pps_pool.tile([128, 512], f32, tag="ps")
                                for mc in range(MC):
                                    nc.tensor.matmul(
                                        ps[:],
                                        vt_h[half][:, mc,
                                                   128 * hl:128 * hl + 128],
                                        wv_st[:, mc, :], start=(mc == 0),
                                        stop=(mc == MC - 1))
                                vevac[(half, cb, hl)] = nc.vector.tensor_add(
                                    vhat[:, h, 4 * cb:4 * cb + 4, :], ps[:],
                                    bvb[:, 512 * cb:512 * cb + 512])

                # ---- Q projection rb=1 only (rb=0 interleaves into attn) ----
                with nc.named_scope("proj_q"):
                    # half 1 first: it feeds the main (rb=1) phase, and the
                    # first-loaded tile gets the earlier-released stage slot
                    qt_h = [None, None]
                    for half in (1, 0):
                        st = st_pool.tile([128, MC, 512], bf16, tag="st")
                        nc.sync.dma_start(st[:], half_src(qts_d, half, 0, MC))
                        qt_h[half] = st
                    for ct in range(MC):
                        wq_st = w_pool.tile([128, MC, 128], bf16, tag="w")
                        nc.sync.dma_start(wq_st[:], wqt_d.ap()[ct])
                        ps = pps_pool.tile([128, 512], f32, tag="ps")
                        for mc in range(MC):
                            nc.tensor.matmul(ps[:], wq_st[:, mc, :],
                                             qt_h[1][:, mc, :],
                                             start=(mc == 0), stop=(mc == MC - 1))
                        qevac[(1, ct)] = nc.scalar.activation(
                            q_sb[:, ct, 4:8, :], ps[:], AF.Identity,
                            bias=bq_sb[:, ct:ct + 1], scale=SCALE)

            # ---- attention + output projection (+ Q proj rb=0) ----
            with ExitStack() as aouter:
                x_pool = aouter.enter_context(tc.tile_pool(name="xsb", bufs=1))
                wo_pool = aouter.enter_context(tc.tile_pool(name="wo", bufs=4))
                wq2_pool = aouter.enter_context(
                    tc.tile_pool(name="wq2", bufs=2))
                oout_pool = aouter.enter_context(
                    tc.tile_pool(name="oout", bufs=3))
                x_sb = x_pool.tile([128, HPC, 2, 512], bf16)  # [d'][h][j%2][q]

                def emit_otile(ops_pool, j, ot, gate_dep=None):
                    wo_st = wo_pool.tile([128, HPC, 128], bf16, tag="wo")
                    wo_dma = nc.sync.dma_start(wo_st[:], wot_d.ap()[ot])
                    gate(wo_dma, gate_dep, "wo prefetch gate")
                    op = ops_pool.tile([128, 512], f32, tag="op")
                    for h in range(HPC):
                        nc.tensor.matmul(op[:], wo_st[:, h, :],
                                         x_sb[:, h, j % 2, :],
                                         start=(h == 0), stop=(h == HPC - 1))
                    oo = oout_pool.tile([128, 512], bf16, tag="oo")
                    nc.vector.tensor_scalar(oo[:], op[:], bo_sb[:, ot:ot + 1],
                                            None, mybir.AluOpType.add)
                    nc.sync.dma_start(
                        out_d.ap()[128 * ot:128 * ot + 128,
                                   512 * j:512 * j + 512], oo[:])

                with ExitStack() as ph, nc.named_scope("attn"):
                    exp_pool = ph.enter_context(
                        tc.tile_pool(name="expp", bufs=6))
                    tree_pool = ph.enter_context(
                        tc.tile_pool(name="tree", bufs=2))
                    scps_pool = ph.enter_context(
                        tc.tile_pool(name="scps", bufs=2, space="PSUM"))
                    xps_pool = ph.enter_context(
                        tc.tile_pool(name="xps", bufs=2, space="PSUM"))
                    sps_pool = ph.enter_context(
                        tc.tile_pool(name="sps", bufs=1, space="PSUM"))
                    ops_pool = ph.enter_context(
                        tc.tile_pool(name="ops", bufs=1, space="PSUM"))
                    nrm_pool = ph.enter_context(tc.tile_pool(name="nrm",
                                                             bufs=2))

                    def attn_block(j, h, defer_in):
                        x_ps = xps_pool.tile([128, 512], f32, tag="xps")
                        s_ps = sps_pool.tile([1, 512], f32, tag="sps")
                        acc = tree_pool.tile([128, 2, 512], bf16, tag="acc")
                        tsum = tree_pool.tile([128, 512], bf16, tag="tf")
                        exs = [None] * NP_
                        ys = [None] * 4
                        zs = [None] * 2
                        q_rhs = q_sb[:, 4 * j:4 * j + 4, h, :]

                        def pv_and_sum(tp):
                            ex = exs[tp]
                            for i in range(2):
                                nc.tensor.matmul(
                                    x_ps[:], vhat[:, h, 2 * tp + i, :],
                                    ex[:, i, :], start=(tp == 0 and i == 0),
                                    stop=(tp == NP_ - 1 and i == 1))
                            if tp % 2 == 1:
                                a = tp // 2
                                y = tree_pool.tile([128, 2, 512], bf16,
                                                   tag="y")
                                nc.vector.tensor_add(y[:], exs[tp - 1][:],
                                                     ex[:])
                                ys[a] = y
                            if tp == 3:
                                z = tree_pool.tile([128, 2, 512], bf16,
                                                   tag="z")
                                nc.vector.tensor_add(z[:], ys[0][:], ys[1][:])
                                zs[0] = z
                            elif tp == NP_ - 1:
                                z = tree_pool.tile([128, 2, 512], bf16,
                                                   tag="z")
                                nc.vector.tensor_add(z[:], ys[2][:], ys[3][:])
                                zs[1] = z
                                nc.vector.tensor_add(acc[:, :, :], zs[0][:],
                                                     zs[1][:])
                                nc.vector.tensor_add(tsum[:], acc[:, 0, :],
                                                     acc[:, 1, :])

                        def finisher():
                            nc.tensor.matmul(s_ps[:], onescol[:], tsum[:],
                                             start=True, stop=True)
                            rec = nrm_pool.tile([1, 512], f32, tag="rec")
                            nc.vector.reciprocal_approx_fast(rec[:], s_ps[:])
                            bcast = nrm_pool.tile([128, 512], f32, tag="bc")
                            nc.gpsimd.partition_broadcast(bcast[:], rec[:])
                            nc.vector.tensor_mul(x_sb[:, h, j % 2, :],
                                                 x_ps[:], bcast[:])

                        for tp in range(NP_):
                            sc = scps_pool.tile([128, 2, 512], f32, tag="sc")
                            for i in range(2):
                                tk = 2 * tp + i
                                nc.tensor.matmul(sc[:, i, :],
                                                 khat[:, tk, h, :],
                                                 q_rhs, start=True, stop=True)
                            ex = exp_pool.tile([128, 2, 512], bf16, tag="ex")
                            nc.scalar.activation(ex[:], sc[:], AF.Exp,
                                                 scale=1.0)
                            exs[tp] = ex
                            if tp == 0:
                                for fn in defer_in:
                                    fn()
                            if tp >= 2:
                                pv_and_sum(tp - 2)
                        return [lambda: pv_and_sum(NP_ - 2),
                                lambda: pv_and_sum(NP_ - 1), finisher]

                    HORD = [4, 5, 6, 7, 0, 1, 2, 3]
                    defer = []
                    for j in range(4):
                        for hp, h in enumerate(HORD):
                            defer = attn_block(j, h, defer)
                            if j == 0:
                                # Q projection rb=0, two ct-groups per block
                                for ct in (2 * hp, 2 * hp + 1):
                                    wq_st = wq2_pool.tile([128, MC, 128], bf16,
                                                          tag="w")
                                    nc.sync.dma_start(wq_st[:],
                                                      wqt_d.ap()[ct])
                                    ps = ops_pool.tile([128, 512], f32,
                                                       tag="op")
                                    for mc in range(MC):
                                        nc.tensor.matmul(
                                            ps[:], wq_st[:, mc, :],
                                            qt_h[0][:, mc, :],
                                            start=(mc == 0),
                                            stop=(mc == MC - 1))
                                    qevac[(0, ct)] = nc.scalar.activation(
                                        q_sb[:, ct, 0:4, :], ps[:],
                                        AF.Identity,
                                        bias=bq_sb[:, ct:ct + 1], scale=SCALE)
                            else:
                                gd = qevac[(0, 15)] if j == 1 else None
                                emit_otile(ops_pool, j - 1, 2 * hp, gd)
                                emit_otile(ops_pool, j - 1, 2 * hp + 1, gd)
                    for fn in defer:
                        fn()

                with ExitStack() as tl:
                    ops2_pool = tl.enter_context(
                        tc.tile_pool(name="ops2", bufs=2, space="PSUM"))
                    for ot in range(MC):
                        emit_otile(ops2_pool, 3, ot)

    nc.compile()
    return nc


def _prep_shared(Wq, Wk, Wv, Wo, bq, bk, bv, bo):
    bf16 = _bf16()
    Wq = np.asarray(Wq, np.float32)
    Wk = np.asarray(Wk, np.float32)
    Wv = np.asarray(Wv, np.float32)
    Wo = np.asarray(Wo, np.float32)
    wqt = np.ascontiguousarray(
        Wq.reshape(MC, 128, MC, 128).transpose(0, 3, 2, 1)).astype(bf16)
    wkt = np.ascontiguousarray(
        Wk.reshape(MC, 128, MC, 128).transpose(0, 3, 2, 1)).astype(bf16)
    wvt = np.ascontiguousarray(
        Wv.reshape(4, 512, MC, 128).transpose(0, 3, 2, 1)).astype(bf16)
    wo4 = Wo.reshape(MC, 128, MC, 128)
    wot = [np.ascontiguousarray(
        wo4[:, :, 8 * half:8 * half + 8, :].transpose(0, 3, 2, 1)).astype(bf16)
        for half in range(2)]
    bias = np.empty((128, 3 * MC), np.float32)
    bias[:, 0:MC] = (np.asarray(bq, np.float32) * SCALE).reshape(MC, 128).T
    bias[:, MC:2 * MC] = np.asarray(bk, np.float32).reshape(MC, 128).T
    bias[:, 2 * MC:3 * MC] = np.asarray(bo, np.float32).reshape(MC, 128).T
    bvr = np.asarray(bv, np.float32).reshape(1, D).copy()
    return wqt, wkt, wvt, wot, bias, bvr


def kernel(Q, K, V, Wq, bq, Wk, bk, Wv, bv, Wo, bo, num_heads):
    global last_results
    assert int(num_heads) == H

    from concourse.bass_utils import run_bass_kernel_spmd

    if "nc" not in _cache:
        _cache["nc"] = _build()
    nc = _cache["nc"]

    bf16 = _bf16()
    Q = np.asarray(Q, np.float32)
    K = np.asarray(K, np.float32)
    V = np.asarray(V, np.float32)
    wqt, wkt, wvt, wot, bias, bvr = _prep_shared(
        Wq, Wk, Wv, Wo, bq, bk, bv, bo)

    in_maps = []
    for c in range(NC_):
        b, half = divmod(c, 2)
        r0 = RPC * half
        in_maps.append({
            "qts": np.ascontiguousarray(Q[b].T[:, r0:r0 + RPC]).astype(bf16)
            .reshape(MC, 128, RPC),
            "kts": np.ascontiguousarray(K[b].T[:, r0:r0 + RPC]).astype(bf16)
            .reshape(MC, 128, RPC),
            "vts": np.ascontiguousarray(V[b].T[:, r0:r0 + RPC]).astype(bf16)
            .reshape(MC, 128, RPC),
            "wqt": wqt, "wkt": wkt, "wvt": wvt, "wot": wot[half],
            "bias": bias, "bvr": bvr,
        })

    res = run_bass_kernel_spmd(nc, in_maps, core_ids=list(range(NC_)))
    last_results = res

    out = np.empty((B, S, D), np.float32)
    for b in range(B):
        oT = (np.asarray(res.results[2 * b]["out"], np.float32)
              + np.asarray(res.results[2 * b + 1]["out"], np.float32))
        # oT[o, pi], pi = 128*t + u ; s = 16*u + t
        out[b] = oT.reshape(D, 16, 128).transpose(2, 1, 0).reshape(S, D)
    return out
